# revision 3
# baseline (speedup 1.0000x reference)
"""Bass/Trainium2 kernel for BoundaryAwareDiceLoss (data-parallel over 8 NeuronCores).

Math (matches the jax reference):
  dice = 1 - (2*sum(p*t) + 1e-5) / (sum(p) + sum(t) + 1e-5)
  bce  = -mean(t*log(p) + (1-t)*log(1-p))      [t in {0,1}]
  bmask = fg & (any of the 6 axis neighbors (b+-1, h+-1, w+-1), edge-clamped,
          is background)
  out  = dice + 10 * bce * mean(bmask)

Key reformulation: with centered uploads pp = pred-0.5 (bf16) and
tq = target-0.5 in {+-0.5} (fp8e4, exact):
  per-pixel BCE term:  t*log(p)+(1-t)*log(1-p) = log(q),  q = 2*pp*tq + 0.5
  so  sum(log q) = one fused DVE scalar_tensor_tensor (v = pp*tq, accum=s_v)
  followed by one ScalarE Ln(2v + 0.5001) pass with accum (the +1e-4 bias
  also self-clamps q away from 0; error ~1e-3 on bce, final rel err ~4e-4).
  dice needs only S = sum(p)+sum(t) and s_v:
  sum(p*t) = s_v + S/2 - N/4.
  sum(pp) / sum(tq) come from ones-weight matmuls summed in PSUM.

Boundary counting on PE in fp8 DoubleRow perf mode (2 shift-operands per
matmul via a 2-element j-axis in the rhs AP):
  total7 = sum of (self + 6 clamped neighbors) of tq -> c ones give (2c-7)/2,
  non-boundary fg <=> total7 = 3.5. Per 128-row block k the 7 contributions
  are 3-4 DoubleRow matmuls: (w-1|tri(self,h+-1)), (b-1|w+1), cross-k fixes
  paired with b+1. PSUM drains (is_ge 3.0 on DVE / Relu(x-3) on ACT, both
  with accum_out) produce the count. sum(bmask) = sum(t) - count.

Per-core: 4 owned batch planes + 2 halo planes; w-clamp pad columns are baked
into the host-side tq layout (col 3 = w0, col 516 = w511). Host combine in
float64.
"""

import numpy as np
import ml_dtypes

BF16 = ml_dtypes.bfloat16
FP8 = ml_dtypes.float8_e4m3

B_TOTAL, C, H, W = 32, 1, 512, 512
NCORES = 8
B_OWN = B_TOTAL // NCORES  # 4
P = 128
K = H // P  # 4
SLOTS = B_OWN + 2  # 6: [halo_lo, b0..b3, halo_hi]
CW = 520  # padded slot width: [0,0,0, lpad, w0..w511, rpad, 0,0,0]
DCOL = 4  # first data column
NPIX = float(B_TOTAL * C * H * W)
WEIGHT = 10.0
SMOOTH = 1e-5
LNBIAS = 0.5001  # 0.5 + 1e-4 self-clamp

# chunk drain engine assignment (8 chunks of [128,1024] psum per rep)
DVE_CHUNKS = (0, 3, 6)  # rest go to ACT (relu, scaled 0.5/hit)

_CACHE = {}


def _tri(kind):
    w = np.zeros((P, P), np.float32)
    for m in range(P):
        for d in (-1, 0, 1):
            if 0 <= m + d < P:
                w[m + d, m] = 1.0
    if kind == 0:
        w[0, 0] += 1.0  # k==0: h-1 clamps to h=0 (self)
    elif kind == 3:
        w[P - 1, P - 1] += 1.0  # k==K-1: h+1 clamps to h=511
    return w


def _make_weights():
    I = np.eye(P, dtype=np.float32)
    Z = np.zeros((P, P), np.float32)
    up = np.zeros((P, P), np.float32)
    up[P - 1, 0] = 1.0  # out[0] += prev-k row 127
    dn = np.zeros((P, P), np.float32)
    dn[0, P - 1] = 1.0  # out[127] += next-k row 0
    # pair matrices [j, m]: j0 applied to rhs slice 0, j1 to slice 1
    pairs = {
        "A0": (I, _tri(0)),
        "Am": (I, _tri(1)),
        "A3": (I, _tri(3)),
        "B": (I, I),
        "C": (up, I),
        "D": (I, dn),
        "E": (dn, Z),
    }
    names = list(pairs)
    wnp = np.zeros((P, len(names), 2, P), np.float32)  # [p, i, j, m]
    for i, nm in enumerate(names):
        wnp[:, i, 0, :] = pairs[nm][0]
        wnp[:, i, 1, :] = pairs[nm][1]
    return names, wnp.reshape(P, len(names) * 2 * P).astype(FP8)


def _build_nc(nrep=1):
    import concourse.bacc as bacc
    import concourse.mybir as mybir
    from concourse.tile import TileContext

    dt = mybir.dt
    alu = mybir.AluOpType
    act = mybir.ActivationFunctionType
    DR = mybir.MatmulPerfMode.DoubleRow

    wnames, wpairs = _make_weights()
    WI = {nm: i for i, nm in enumerate(wnames)}
    NW = len(wnames)

    nc = bacc.Bacc("TRN2", target_bir_lowering=False)
    pp_d = nc.dram_tensor("pp", [K, P, B_OWN * W], dt.bfloat16, kind="ExternalInput")
    tq_d = nc.dram_tensor("tq", [K, P, SLOTS * CW], dt.float8e4, kind="ExternalInput")
    out_d = nc.dram_tensor("out", [P, 13], dt.float32, kind="ExternalOutput")
    os_d = nc.dram_tensor("osum", [1, 2], dt.float32, kind="ExternalOutput")
    wts_d = nc.inline_tensor(wpairs, name="wts")
    tones_d = nc.inline_tensor(np.ones((P, 2 * 32), FP8), name="tones")
    pones_d = nc.inline_tensor(np.ones((P, 1), BF16), name="pones")

    with TileContext(nc) as tc:
        with (
            tc.tile_pool(name="big", bufs=1) as big,
            tc.tile_pool(name="db", bufs=2) as db,
            tc.tile_pool(name="ps", bufs=3, space="PSUM") as psp,
            tc.tile_pool(name="pso", bufs=1, space="PSUM") as pso,
        ):
            lnscrap = big.tile([P, K * B_OWN * W], dt.bfloat16)
            scrapD = big.tile([P, 1024], dt.bfloat16)
            scrapA = big.tile([P, 1024], dt.bfloat16)
            bq = big.tile([P, 1], dt.float32)
            bm3 = big.tile([P, 1], dt.float32)
            nc.vector.memset(bq[:], LNBIAS)
            nc.vector.memset(bm3[:], -3.0)

            for _rep in range(nrep):
                tq = db.tile([P, K * SLOTS * CW], dt.float8e4, name="tq", tag="tq")
                pp = db.tile([P, K * B_OWN * W], dt.bfloat16, name="pp", tag="pp")
                v = db.tile([P, K * B_OWN * W], dt.float16, name="v", tag="v")
                wsb = db.tile([P, NW * 2 * P], dt.float8e4, name="wsb", tag="wsb")
                tosb = db.tile([P, 2 * 32], dt.float8e4, name="tosb", tag="tosb")
                posb = db.tile([P, 1], dt.bfloat16, name="posb", tag="posb")
                racc = db.tile([P, 13], dt.float32, name="racc", tag="racc")
                osum = db.tile([1, 2], dt.float32, name="osum", tag="osum")

                nc.sync.dma_start(out=wsb[:], in_=wts_d[:])
                nc.sync.dma_start(out=tosb[:], in_=tones_d[:])
                nc.sync.dma_start(out=posb[:], in_=pones_d[:])
                for k in range(K):
                    nc.sync.dma_start(
                        out=tq[:, k * SLOTS * CW : (k + 1) * SLOTS * CW],
                        in_=tq_d[k],
                    )
                    nc.sync.dma_start(
                        out=pp[:, k * B_OWN * W : (k + 1) * B_OWN * W],
                        in_=pp_d[k],
                    )

                wv = wsb[:].rearrange("p (i j m) -> p i j m", i=NW, j=2)

                def rhs_pair(off, jstride):
                    ap = tq[:].copy()
                    ap.ap = mybir.VecI64Pair(
                        [list(tq[:].ap[0]), [jstride, 2], [1, W]]
                    )
                    ap.offset = tq[:].offset + off
                    return ap

                def base(k, s, c):
                    return k * SLOTS * CW + s * CW + c

                # boundary matmuls: per k two [128,1024] psum chunks
                # (units b0,b1 and b2,b3), weight-grouped to limit reloads
                chunks = []
                for k in range(K):
                    ch = [
                        psp.tile([P, 1024], dt.float32, name="ch", tag="ch")
                        for _ in range(2)
                    ]

                    def outp(b):
                        return ch[b // 2][:, (b % 2) * 512 : (b % 2) * 512 + 512]

                    groups = [(WI["A0" if k == 0 else ("A3" if k == K - 1 else "Am")],
                               lambda b: rhs_pair(base(k, b + 1, 3), 1), True)]
                    groups.append((WI["B"],
                                   lambda b: rhs_pair(base(k, b, DCOL), CW + 1), False))
                    if k == 0:
                        groups.append((WI["D"],
                                       lambda b: rhs_pair(base(0, b + 2, DCOL), (SLOTS - 1) * CW), False))
                    else:
                        groups.append((WI["C"],
                                       lambda b: rhs_pair(base(k - 1, b + 1, DCOL), (SLOTS + 1) * CW), False))
                        if k < K - 1:
                            groups.append((WI["E"],
                                           lambda b: rhs_pair(base(k + 1, b + 1, DCOL), 0), False))
                    ng = len(groups)
                    for gi, (wi, rf, is_start) in enumerate(groups):
                        for b in range(B_OWN):
                            nc.tensor.matmul(
                                outp(b), wv[:, wi], rf(b),
                                start=is_start, stop=(gi == ng - 1),
                                perf_mode=DR,
                            )
                    chunks.extend(ch)

                    # drain the two finished chunks of this k
                    for half in range(2):
                        ci = 2 * k + half
                        cht = ch[half]
                        if ci in DVE_CHUNKS:
                            nc.vector.tensor_scalar(
                                out=scrapD[:], in0=cht[:],
                                scalar1=3.0, scalar2=0.0,
                                op0=alu.is_ge, op1=alu.add,
                                accum_out=racc[:, 5 + ci : 6 + ci],
                            )
                        else:
                            nc.scalar.activation(
                                out=scrapA[:], in_=cht[:], func=act.Relu,
                                bias=bm3[:, 0:1], scale=1.0,
                                accum_out=racc[:, 5 + ci : 6 + ci],
                            )

                # elementwise: v = pp*tq (accum s_v per k), then Ln(2v+0.5001)
                ppv = pp[:].rearrange("p (k b w) -> p k b w", k=K, b=B_OWN)
                vv = v[:].rearrange("p (k b w) -> p k b w", k=K, b=B_OWN)
                for k in range(K):
                    tqo = tq[:].copy()
                    tqo.ap = mybir.VecI64Pair(
                        [list(tq[:].ap[0]), [CW, B_OWN], [1, W]]
                    )
                    tqo.offset = tq[:].offset + k * SLOTS * CW + CW + DCOL
                    nc.vector.scalar_tensor_tensor(
                        out=vv[:, k], in0=ppv[:, k], scalar=0.0, in1=tqo,
                        op0=alu.add, op1=alu.mult,
                        accum_out=racc[:, k : k + 1],
                    )
                nc.scalar.activation(
                    out=lnscrap[:], in_=v[:], func=act.Ln,
                    bias=bq[:, 0:1], scale=2.0,
                    accum_out=racc[:, 4:5],
                )

                # ones-sums on PE (emitted after chunk mms to avoid
                # head-blocking the PE queue across reps)
                onst = pso.tile([32, 512], dt.float32, name="onst", tag="onst")
                onsp = pso.tile([1, 512], dt.float32, name="onsp", tag="onsp")
                tov = tosb[:].rearrange("p (j m) -> p j m", j=2)
                cnt = 0
                for k in range(K):
                    for pair0 in (1, 3):
                        nc.tensor.matmul(
                            onst[:], tov, rhs_pair(base(k, pair0, DCOL), CW),
                            start=(cnt == 0), stop=(cnt == 2 * K - 1),
                            perf_mode=DR,
                        )
                        cnt += 1
                cnt = 0
                for k in range(K):
                    for u in range(B_OWN):
                        nc.tensor.matmul(
                            onsp[:], posb[:, 0:1],
                            pp[:, (k * B_OWN + u) * W : (k * B_OWN + u + 1) * W],
                            start=(cnt == 0), stop=(cnt == K * B_OWN - 1),
                        )
                        cnt += 1
                nc.vector.tensor_scalar(
                    out=scrapD[0:1, 0:512], in0=onst[0:1, :],
                    scalar1=0.0, scalar2=0.0, op0=alu.add, op1=alu.add,
                    accum_out=osum[:, 0:1],
                )
                nc.vector.tensor_scalar(
                    out=scrapD[0:1, 0:512], in0=onsp[:],
                    scalar1=0.0, scalar2=0.0, op0=alu.add, op1=alu.add,
                    accum_out=osum[:, 1:2],
                )

                nc.sync.dma_start(out=out_d[:], in_=racc[:])
                nc.sync.dma_start(out=os_d[:], in_=osum[:])

    nc.compile()
    return nc


def _get_nc(nrep=1):
    if nrep not in _CACHE:
        _CACHE[nrep] = _build_nc(nrep)
    return _CACHE[nrep]


def _shard_inputs(pred, target):
    pred = np.asarray(pred, dtype=np.float32).reshape(B_TOTAL, H, W)
    tgt = np.asarray(target, dtype=np.float32).reshape(B_TOTAL, H, W)
    tq_full = np.where(tgt > 0.5, np.float32(0.5), np.float32(-0.5))
    in_maps = []
    for c in range(NCORES):
        b0 = c * B_OWN
        # pp: [K, P, B_OWN, W] bf16 (pred - 0.5)
        pc = pred[b0 : b0 + B_OWN].reshape(B_OWN, K, P, W) - np.float32(0.5)
        pc = np.ascontiguousarray(pc.transpose(1, 2, 0, 3)).astype(BF16)
        # tq: [K, P, SLOTS, CW] fp8 with w-clamp pads
        lo = max(b0 - 1, 0)
        hi = min(b0 + B_OWN, B_TOTAL - 1)
        idx = [lo] + list(range(b0, b0 + B_OWN)) + [hi]
        tc = tq_full[idx].reshape(SLOTS, K, P, W).transpose(1, 2, 0, 3)
        buf = np.zeros((K, P, SLOTS, CW), np.float32)
        buf[..., DCOL : DCOL + W] = tc
        buf[..., DCOL - 1] = tc[..., 0]
        buf[..., DCOL + W] = tc[..., W - 1]
        in_maps.append(
            {
                "pp": pc.reshape(K, P, B_OWN * W),
                "tq": buf.astype(FP8).reshape(K, P, SLOTS * CW),
            }
        )
    return in_maps


def _combine(results):
    s_v = s_lq = s_cnt_dve = s_cnt_act = s_tq = s_pp = 0.0
    for r in results:
        out = np.asarray(r["out"], np.float64)
        osum = np.asarray(r["osum"], np.float64)
        s_v += out[:, 0:4].sum()
        s_lq += out[:, 4].sum()
        for ci in range(8):
            if ci in DVE_CHUNKS:
                s_cnt_dve += out[:, 5 + ci].sum()
            else:
                s_cnt_act += out[:, 5 + ci].sum()
        s_tq += osum[0, 0]
        s_pp += osum[0, 1]
    N = NPIX
    s_r = s_cnt_dve + 2.0 * s_cnt_act
    s_t = s_tq + 0.5 * N
    s_p = s_pp + 0.5 * N
    S = s_p + s_t
    s_pt = s_v + 0.5 * S - 0.25 * N
    dice = 1.0 - (2.0 * s_pt + SMOOTH) / (S + SMOOTH)
    bce = -s_lq / N
    mb = (s_t - s_r) / N
    return np.asarray(dice + WEIGHT * bce * mb, dtype=np.float32)


TRACE = False
LAST_RESULTS = None


def kernel(pred, target):
    global LAST_RESULTS
    from concourse.bass_utils import run_bass_kernel_spmd

    in_maps = _shard_inputs(pred, target)
    nc = _get_nc()
    res = run_bass_kernel_spmd(
        nc, in_maps, core_ids=list(range(NCORES)), trace=TRACE
    )
    LAST_RESULTS = res
    return _combine([r for r in res.results])
